# revision 64
# baseline (speedup 1.0000x reference)
"""Bass/Trainium2 kernel for nn_MultiHeadAttention (T5-style rel-bias causal MHA).

Sharding: 8 cores = 2 batches x 4 head-groups (4 heads of 64 dims each).
Each core: projects q/k/v for its 256 proj rows (bf16 inputs/weights, full
PE speed), runs causal attention with the T5 relative bias applied either
as a constant exp-bias (bucket-31-saturated blocks) or multiplicatively on
the Vector engine (es *= exp(band), near-diagonal blocks; the exp-band
table is trimmed to the 1024 band columns actually touched), and computes
a partial out-projection. Host sums the 4 partials per batch.

Loop order is q-section-outer so the out-projection of section 0 overlaps
the attention of section 1, keeping the PE continuously busy (DVFS ramp).
"""
import math
import sys

sys.path.insert(0, "/opt/trn_rl_repo")

import numpy as np
import ml_dtypes

from concourse import bacc
import concourse.mybir as mybir
import concourse.tile as tile
from concourse.bass_utils import run_bass_kernel_spmd

F32 = mybir.dt.float32
F32R = mybir.dt.float32r
BF16 = mybir.dt.bfloat16
Exp = mybir.ActivationFunctionType.Exp
Identity = mybir.ActivationFunctionType.Identity
MUL = mybir.AluOpType.mult

B, L, D = 2, 2048, 1024
H, HD = 16, 64
NUM_BUCKETS, MAX_DISTANCE = 32, 128
HPC = 4  # heads per core
MPC = HPC * HD  # 256 proj rows per core
N_CORES = 8
NEG = -60.0  # additive mask value (exp(-60+s) == 0 in practice)
EBX0 = 1664  # first band-table column used by any non-const block
EBW = 1024  # band-table width actually touched

# live k-blocks per (qs, j) half, and the k-blocks whose bias saturates
# to bucket 31 (distance >= 113 everywhere in the half)
LIVE = {(0, 0): 4, (0, 1): 8, (1, 0): 12, (1, 1): 16}
CONSTK = {
    (0, 0): frozenset(),
    (0, 1): frozenset(range(3)),
    (1, 0): frozenset(range(7)),
    (1, 1): frozenset(range(11)),
}

last_results = None  # BassKernelResults of the most recent run (for profiling)
_cached = None


def _bf16(x: np.ndarray) -> np.ndarray:
    return np.ascontiguousarray(x).astype(ml_dtypes.bfloat16)


def _bucket(rp: np.ndarray) -> np.ndarray:
    """T5 relative position bucket, mirrors the reference exactly."""
    sign = (rp > 0).astype(np.int32)
    n = np.abs(rp)
    max_exact = NUM_BUCKETS // 2
    n_safe = np.maximum(n, 1).astype(np.float32)
    vil = max_exact + (
        np.log(n_safe / max_exact)
        / math.log(MAX_DISTANCE / max_exact)
        * (NUM_BUCKETS - max_exact)
    ).astype(np.int32)
    vil = np.minimum(vil, NUM_BUCKETS - 1)
    buckets = np.where(n < max_exact, n, vil) + sign * max_exact
    return np.clip(buckets, 0, NUM_BUCKETS - 1)


def _build():
    nc = bacc.Bacc(trn_type="TRN2")

    qT_in = nc.dram_tensor("qT_in", [D, L], BF16, kind="ExternalInput")
    kT_in = nc.dram_tensor("kT_in", [D, L], BF16, kind="ExternalInput")
    vT_in = nc.dram_tensor("vT_in", [D, L], BF16, kind="ExternalInput")
    wq_in = nc.dram_tensor("wq_in", [128, 8, MPC], BF16, kind="ExternalInput")
    wk_in = nc.dram_tensor("wk_in", [128, 8, MPC], BF16, kind="ExternalInput")
    wv_in = nc.dram_tensor("wv_in", [128, 8, MPC], BF16, kind="ExternalInput")
    wo_in = nc.dram_tensor("wo_in", [128, 2, D], BF16, kind="ExternalInput")
    bq_in = nc.dram_tensor("bq_in", [128, 2], F32, kind="ExternalInput")
    bk_in = nc.dram_tensor("bk_in", [128, 2], F32, kind="ExternalInput")
    eb_in = nc.dram_tensor("eb_in", [128, HPC, EBW], BF16, kind="ExternalInput")
    c31_in = nc.dram_tensor("c31_in", [128, HPC], F32, kind="ExternalInput")
    outT = nc.dram_tensor("outT", [D, L], BF16, kind="ExternalOutput")

    with tile.TileContext(nc) as tc:
        with tc.tile_pool(name="res", bufs=1) as pr:
            # (bq/bk/c31 DMAs are issued inside the proj section, behind
            # the first input chunk, so they don't delay the first matmul)
            bq = pr.tile([128, 2], F32)
            bk = pr.tile([128, 2], F32)
            c31 = pr.tile([128, HPC], F32)
            ones_v = pr.tile([1, HD], F32R)
            nc.vector.memset(ones_v[:].bitcast(F32), 1.0)
            # dummy broadcast: pull the gpsimd ucode library load into the
            # startup DMA-wait window instead of mid-attention
            gwarm = pr.tile([4, HD], F32R)
            nc.gpsimd.partition_broadcast(gwarm[:], ones_v[:])
            # tiny warm tile memset FIRST so PE warmup matmuls can start
            # before the big qTz memsets finish
            wz = pr.tile([128, 512], BF16)
            nc.vector.memset(wz[:], 0.0)
            # warm the ACT exp table early, off the critical path
            warm = pr.tile([1, 2], F32)
            nc.vector.memset(warm[:], 0.0)
            nc.scalar.activation(warm[:], warm[:], Exp)

            eb = pr.tile([128, HPC, EBW], BF16)
            wo = pr.tile([128, 2, D], BF16)
            qTz = []
            for hh in range(HPC):
                t = pr.tile([128, L], BF16, name=f"qtz{hh}")
                nc.vector.memset(t[:], 0.0)
                qTz.append(t)
            kTt = [pr.tile([128, L], BF16, name=f"kt{mm}") for mm in range(2)]
            vx = pr.tile([128, 16, HPC, HD + 1], BF16)
            nc.vector.memset(vx[:, :, :, HD], 1.0)
            y_norm_qs = [
                pr.tile([128, 2, 1024], BF16, name=f"yn{qq}") for qq in range(2)
            ]

            # ---------------- projections ----------------
            with (
                tc.tile_pool(name="proj", bufs=1) as pp,
                tc.tile_pool(name="stg", bufs=12) as pstg,
                tc.tile_pool(name="ppsum", bufs=8, space="PSUM") as pps,
            ):
                dma_engs = [nc.sync, nc.scalar]
                wq = pp.tile([128, 8, MPC], BF16)
                # first half only — the first matmuls need just chunk 0
                nc.scalar.dma_start(wq[:, 0:4, :], wq_in[:, 0:4, :])
                wk = pp.tile([128, 8, MPC], BF16)
                wv = pp.tile([128, 8, MPC], BF16)

                # PE DVFS warmup: dummy matmuls on the zeroed qTz while the
                # first input chunks are still in flight
                wps = pps.tile([128, 512], F32, tag="qk", name="warm_ps")
                for _ in range(28):
                    nc.tensor.matmul(
                        wps[:], wz[:, 0:128], wz[:],
                        start=True, stop=True,
                    )

                # q/k: transposed locals [m, l] = W_c @ x.T (+bias).
                # Two m-waves of 4 psum banks each: wave B's matmuls overlap
                # wave A's evacs on disjoint banks (no drain stall between
                # phases).
                wave_ps = {}

                def _qkp(m, n):
                    if (m, n) not in wave_ps:
                        wave_ps[(m, n)] = pps.tile(
                            [128, 512], F32, tag="qk", name=f"qkp{m}{n}"
                        )
                    return wave_ps[(m, n)][:]

                def _qk_evac(m, dst, b_sb):
                    for n in range(4):
                        src_ap = wave_ps.pop((m, n))[:]
                        if dst is None:
                            for sub in range(2):
                                pb = 64 * sub
                                d_ap = qTz[2 * m + sub][
                                    pb : pb + 64, 512 * n : 512 * n + 512
                                ]
                                b_ap = b_sb[pb : pb + 64, m : m + 1]
                                if (n + sub) % 2 == 0:
                                    nc.vector.tensor_scalar_add(
                                        d_ap, src_ap[pb : pb + 64, :], b_ap
                                    )
                                else:
                                    nc.scalar.activation(
                                        d_ap, src_ap[pb : pb + 64, :],
                                        Identity, bias=b_ap,
                                    )
                        else:
                            d_ap = dst[m][:, 512 * n : 512 * n + 512]
                            b_ap = b_sb[:, m : m + 1]
                            if n % 2 == 0:
                                nc.vector.tensor_scalar_add(d_ap, src_ap, b_ap)
                            else:
                                nc.scalar.activation(
                                    d_ap, src_ap, Identity, bias=b_ap
                                )

                for src_d, w_sb, b_sb, dst in (
                    (qT_in, wq, bq, None),
                    (kT_in, wk, bk, kTt),
                ):
                    stgs = []
                    for kc in range(8):
                        stg = pstg.tile([128, L], BF16, tag="stage")
                        off = 1 if dst is None else 0
                        if dst is None and kc == 0:
                            nc.scalar.dma_start(wq[:, 4:8, :], wq_in[:, 4:8, :])
                            nc.sync.dma_start(bq[:], bq_in[:])
                            nc.sync.dma_start(bk[:], bk_in[:])
                            nc.sync.dma_start(c31[:], c31_in[:])
                        eng = dma_engs[(kc + off) % len(dma_engs)]
                        eng.dma_start(
                            stg[:], src_d[128 * kc : 128 * kc + 128, :]
                        )
                        stgs.append(stg)
                        # wave 0 consumes chunks as they land
                        for n in range(4):
                            nc.tensor.matmul(
                                _qkp(0, n),
                                w_sb[:, kc, 0:128],
                                stg[:, 512 * n : 512 * n + 512],
                                start=(kc == 0),
                                stop=(kc == 7),
                            )
                    _qk_evac(0, dst if dst is not None else None, b_sb)
                    for kc in range(8):
                        for n in range(4):
                            nc.tensor.matmul(
                                _qkp(1, n),
                                w_sb[:, kc, 128:256],
                                stgs[kc][:, 512 * n : 512 * n + 512],
                                start=(kc == 0),
                                stop=(kc == 7),
                            )
                    _qk_evac(1, dst, b_sb)
                    if dst is None:
                        nc.scalar.dma_start(wk[:], wk_in[:])
                    else:
                        nc.scalar.dma_start(wv[:], wv_in[:])
                # wo + exp-band tables arrive during v-proj
                nc.scalar.dma_start(wo[:], wo_in[:])
                nc.scalar.dma_start(eb[:], eb_in[:])

                # v: natural layout [l, m]; lhsT = staged vT chunks
                stgv = []
                for kc in range(8):
                    s = pstg.tile([128, L], BF16, tag="stage")
                    eng = dma_engs[kc % len(dma_engs)]
                    eng.dma_start(s[:], vT_in[128 * kc : 128 * kc + 128, :])
                    stgv.append(s)
                for grp in range(2):
                    psv = [
                        pps.tile([128, MPC], F32, tag="qk", name=f"vps{i}")
                        for i in range(8)
                    ]
                    for kc in range(8):
                        for i in range(8):
                            li = grp * 8 + i
                            nc.tensor.matmul(
                                psv[i][:],
                                stgv[kc][:, 128 * li : 128 * li + 128],
                                wv[:, kc, :],
                                start=(kc == 0),
                                stop=(kc == 7),
                            )
                    for i in range(8):
                        li = grp * 8 + i
                        eng = (nc.scalar.copy, nc.vector.tensor_copy)[i % 2]
                        eng(
                            vx[:, li, :, 0:HD],
                            psv[i][:].rearrange("p (h d) -> p h d", h=HPC),
                        )

            # ---------------- attention + out-projection ----------------
            # Scores use full K=128 contraction: lhsT carries BOTH heads of
            # the m-tile; the zero rows of qTz kill the other head exactly.
            # (K=128 keeps the PE activity monitor warm; K=64 never warms.)
            with (
                tc.tile_pool(name="es", bufs=5) as pes,
                tc.tile_pool(name="misc", bufs=2) as pmisc,
                tc.tile_pool(name="ost", bufs=4) as post,
                tc.tile_pool(name="spsum", bufs=2, space="PSUM") as psc,
                tc.tile_pool(name="ypsum", bufs=1, space="PSUM") as psy,
                tc.tile_pool(name="apsum", bufs=1, space="PSUM") as pax,
            ):
                pending_norm = None

                def _emit_norm(item):
                    rrow, pb, mt, qsi = item
                    # replicate the recip row on the (idle) GPSIMD engine —
                    # the old K=1 PE matmuls never kept the activity
                    # monitor warm, downclocking the out-projection
                    rep = pmisc.tile([128, 1024], BF16, tag="rep", name="rep")
                    nc.gpsimd.partition_broadcast(rep[:], rrow[:])
                    nc.vector.tensor_tensor(
                        y_norm_qs[qsi][pb : pb + HD, mt, :],
                        y_norm_qs[qsi][pb : pb + HD, mt, :],
                        rep[pb : pb + HD, :],
                        MUL,
                    )

                def _av(yT, qs, h, pend_j):
                    for j, es_t, pki, qo in pend_j:
                        nc.tensor.matmul(
                            yT[:, 512 * j + qo : 512 * j + 512],
                            vx[:, pki, h, :],
                            es_t,
                            start=(pki == 0),
                            stop=(pki == LIVE[(qs, j)] - 1),
                        )

                def _attend(qs, h):
                    nonlocal pending_norm
                    mt, pb, q0 = h // 2, 64 * (h % 2), 1024 * qs
                    yT = psy.tile([HD + 1, 1024], F32, tag="yT", name="yT")
                    pend = []  # AV deferred 3 blocks to hide the exp chain
                    for ki in range(LIVE[(qs, 1)]):
                        halves = [j for j in (0, 1) if ki < LIVE[(qs, j)]]
                        kind = {j: (ki in CONSTK[(qs, j)]) for j in halves}
                        # (trimming the causally-dead staircase columns
                        # measured SLOWER — small-matmul overhead dominates;
                        # keep full-width blocks)
                        qoff = {j: 0 for j in halves}
                        # score ring widened to 3 via the (attention-idle)
                        # aux banks: scores(ki) then waits exp(ki-3), not
                        # exp(ki-2), riding out ACT queue jitter
                        if ki % 3 < 2:
                            sp = psc.tile([128, 1024], F32, tag="score", name="sp")
                        else:
                            sp = pax.tile([128, 1024], F32, tag="aux", name="sp")
                        for j in halves:
                            o = 512 * j + qoff[j]
                            nc.tensor.matmul(
                                sp[:, o : 512 * j + 512],
                                kTt[mt][:, 128 * ki : 128 * ki + 128],
                                qTz[h][:, q0 + o : q0 + 512 * j + 512],
                                start=True,
                                stop=True,
                            )
                        es = pes.tile([128, 1024], BF16, tag="es", name="es")
                        if (
                            len(halves) == 2
                            and kind[0] == kind[1]
                            and qoff[0] == 0
                            and qoff[1] == 0
                        ):
                            bias = c31[:, h : h + 1] if kind[0] else 0.0
                            nc.scalar.activation(es[:], sp[:], Exp, bias=bias)
                        else:
                            for j in halves:
                                o = 512 * j + qoff[j]
                                bias = c31[:, h : h + 1] if kind[j] else 0.0
                                nc.scalar.activation(
                                    es[:, o : 512 * j + 512],
                                    sp[:, o : 512 * j + 512],
                                    Exp,
                                    bias=bias,
                                )
                        pend_j = []
                        for j in halves:
                            o = 512 * j + qoff[j]
                            if not kind[j]:
                                # xi0 = (2048 - 128*ki + q0 + col0) - EBX0
                                xi0 = 384 - 128 * ki + q0 + 512 * j + qoff[j]
                                nc.vector.tensor_tensor(
                                    es[:, o : 512 * j + 512],
                                    es[:, o : 512 * j + 512],
                                    eb[:, h, xi0 : xi0 + 512 - qoff[j]],
                                    MUL,
                                )
                            pend_j.append(
                                (j, es[:, o : 512 * j + 512], ki, qoff[j])
                            )
                        pend.append(pend_j)
                        if len(pend) > 3:
                            _av(yT, qs, h, pend.pop(0))
                    for pend_j in pend:
                        _av(yT, qs, h, pend_j)
                    # kick off the reciprocal chain, then evacuate yT
                    # (unnormalized) into its y_norm slot; the replicate +
                    # in-place multiply for the PREVIOUS section is emitted
                    # now, so the PE never stalls on the recip chain.
                    dcp = pmisc.tile([1, 1024], F32, tag="dcp", name="dcp")
                    nc.vector.tensor_copy(dcp[:], yT[HD : HD + 1, :])
                    dT = pmisc.tile([128, 8], F32, tag="dT", name="dT")
                    nc.sync.dma_start(dT[:], dcp[:])
                    rT = pmisc.tile([128, 8], BF16, tag="rT", name="rT")
                    with nc.allow_low_precision(reason="softmax recip f32r"):
                        nc.vector.reciprocal(rT[:], dT[:])
                    rrow = pmisc.tile([1, 1024], BF16, tag="rrow", name="rrow")
                    nc.sync.dma_start(rrow[:], rT[:])
                    nc.vector.tensor_copy(
                        y_norm_qs[qs][pb : pb + HD, mt, :], yT[0:HD, :]
                    )
                    if pending_norm is not None:
                        _emit_norm(pending_norm)
                    pending_norm = (rrow, pb, mt, qs)

                def _outproj(qs, qis=(0, 1), dve_only=False):
                    for qi in qis:
                        for n in range(8):
                            if n % 2 == 0:
                                # ride the score ring so po evacs never
                                # stall the PE
                                auxt = psc.tile(
                                    [128, 1024], F32, tag="score", name="po"
                                )
                            po = auxt[:, 512 * (n % 2) : 512 * (n % 2) + 512]
                            for c in range(2):
                                nc.tensor.matmul(
                                    po,
                                    wo[:, c, 128 * n : 128 * n + 128],
                                    y_norm_qs[qs][
                                        :, c, 512 * qi : 512 * qi + 512
                                    ],
                                    start=(c == 0),
                                    stop=(c == 1),
                                )
                            ost = post.tile([128, 512], BF16, tag="ost", name="ost")
                            if n % 2 == 0 and not dve_only:
                                nc.scalar.copy(ost[:], po)
                            else:
                                nc.vector.tensor_copy(ost[:], po)
                            # all output DMAs on the idle sync queue — a
                            # scalar-queue doorbell costs ~600ns of ACT time
                            # right when ACT is doing the po evacs
                            nc.sync.dma_start(
                                outT[
                                    128 * n : 128 * n + 128,
                                    1024 * qs + 512 * qi : 1024 * qs + 512 * qi + 512,
                                ],
                                ost[:],
                            )

                for qs, h in (
                    (0, 0), (0, 1), (0, 2), (0, 3),
                    (1, 0), (1, 1), (1, 2), (1, 3),
                ):
                    _attend(qs, h)
                # outproj(0) keeps the PE busy (and the DVFS clock high)
                # while the last section's reciprocal chain flows
                _outproj(0)
                _emit_norm(pending_norm)
                _outproj(1)

    nc.finalize()
    return nc


def _host_tables(rel_emb: np.ndarray):
    """Per-head trimmed exp-band tables; rel_emb is [NUM_BUCKETS, H]."""
    d = np.arange(4095)
    rp = d - 2047  # key - query
    buckets = _bucket(rp)
    # EB[r, xi] = exp(band_pad[4095 + r - (EBX0 + xi)])
    idx = 4095 + np.arange(128)[:, None] - (EBX0 + np.arange(EBW))[None, :]
    ebs = []
    c31s = []
    for h in range(H):
        vals = rel_emb[buckets, h].astype(np.float32)
        vals = np.where(rp > 0, np.float32(NEG), vals)  # causal mask
        band_pad = np.full(4223, NEG, np.float32)
        band_pad[:4095] = vals
        ebs.append(_bf16(np.exp(band_pad[idx])))
        c31s.append(np.float32(rel_emb[31, h]))
    return ebs, c31s


def _numpy_ref(query, key, value, attn_mask, key_padding_mask,
               Wq, bq, Wk, bk, Wv, bv, Wo, bo, rel_emb):
    """Exact numpy fallback for unexpected mask patterns."""
    q = (query @ Wq.T + bq).reshape(B, L, H, HD).transpose(0, 2, 1, 3)
    k = (key @ Wk.T + bk).reshape(B, L, H, HD).transpose(0, 2, 1, 3)
    v = (value @ Wv.T + bv).reshape(B, L, H, HD).transpose(0, 2, 1, 3)
    scores = np.einsum("bhqd,bhkd->bhqk", q, k) / math.sqrt(HD)
    rp = np.arange(L, dtype=np.int64)[None, :] - np.arange(L, dtype=np.int64)[:, None]
    rel = rel_emb[_bucket(rp)].transpose(2, 0, 1)
    scores = scores + rel[None]
    scores = np.where(attn_mask[None, None], scores, -np.inf)
    scores = np.where(key_padding_mask[:, None, None, :], scores, -np.inf)
    scores = scores - scores.max(-1, keepdims=True)
    e = np.exp(scores)
    attn = e / e.sum(-1, keepdims=True)
    out = np.einsum("bhqk,bhkd->bhqd", attn, v)
    out = out.transpose(0, 2, 1, 3).reshape(B, L, D)
    return (out @ Wo.T + bo).astype(np.float32)


def kernel(**inputs) -> np.ndarray:
    global _cached, last_results
    inp = {k: np.asarray(v) for k, v in inputs.items()}
    query, key, value = inp["query"], inp["key"], inp["value"]
    attn_mask, kpm = inp["attn_mask"], inp["key_padding_mask"]
    Wq, bq, Wk, bk = inp["Wq"], inp["bq"], inp["Wk"], inp["bk"]
    Wv, bv, Wo, bo = inp["Wv"], inp["bv"], inp["Wo"], inp["bo"]
    rel_emb = inp["rel_emb"]

    causal = np.array_equal(attn_mask, np.tril(np.ones((L, L), bool)))
    if not (causal and kpm.all()):
        return _numpy_ref(**inp)

    if _cached is None:
        _cached = _build()
    nc = _cached

    ebs, c31s = _host_tables(rel_emb)

    def _rearr_w(w_slice):  # [MPC, D] row-major weights -> [128, 8, MPC]
        arr = np.ascontiguousarray(w_slice.T)  # [D, MPC]
        return _bf16(arr.reshape(8, 128, MPC).transpose(1, 0, 2))

    in_maps = []
    for c in range(N_CORES):
        b, hg = c // HPC, c % HPC
        rows = slice(MPC * hg, MPC * hg + MPC)
        heads = range(HPC * hg, HPC * hg + HPC)
        wo_c = np.ascontiguousarray(Wo[:, rows].T)  # [MPC, D]
        in_maps.append({
            "qT_in": _bf16(query[b].T),
            "kT_in": _bf16(key[b].T),
            "vT_in": _bf16(value[b].T),
            "wq_in": _rearr_w(Wq[rows] / math.sqrt(HD)),
            "wk_in": _rearr_w(Wk[rows]),
            "wv_in": _rearr_w(Wv[rows]),
            "wo_in": _bf16(wo_c.reshape(2, 128, D).transpose(1, 0, 2)),
            "bq_in": np.ascontiguousarray(
                (bq[rows] / math.sqrt(HD)).reshape(2, 128).T.astype(np.float32)
            ),
            "bk_in": np.ascontiguousarray(
                bk[rows].reshape(2, 128).T.astype(np.float32)
            ),
            "eb_in": np.ascontiguousarray(
                np.stack([ebs[h] for h in heads], axis=1)
            ),
            "c31_in": np.tile(
                np.array([c31s[h] for h in heads], np.float32), (128, 1)
            ),
        })

    res = run_bass_kernel_spmd(nc, in_maps, list(range(N_CORES)))
    last_results = res

    bo_eff = (
        bo.astype(np.float64) + bv.astype(np.float64) @ Wo.T.astype(np.float64)
    )
    out = np.empty((B, L, D), np.float32)
    for b in range(B):
        acc = np.zeros((D, L), np.float64)
        for hg in range(HPC):
            acc += res.results[b * HPC + hg]["outT"].astype(np.float64)
        out[b] = (acc.T + bo_eff[None, :]).astype(np.float32)
    return out
